# revision 19
# baseline (speedup 1.0000x reference)
"""Trainium2 Bass kernel for nn_ContrastiveLoss (8-core data-parallel).

reference math (per batch n, L=2048, d=512):
    y[i,j]  = (seg[i] == seg[j])
    d2      = max(||x_i||^2 + ||x_j||^2 - 2 x_i.x_j, 0)
    dist    = sqrt(d2)
    hinge   = max(MARGIN - dist, 0)
    loss    = mean(ALPHA*y*d2 + BETA*(1-y)*hinge^2)

Strategy (one batch per NeuronCore):
  * ALPHA term via exact class aggregation:
        sum_ij y*d2 = sum_c [2*cnt_c*sum_{i in c}||x_i||^2 - 2*||sum_{i in c}x_i||^2]
    computed on-device with one-hot matmuls (bf16 hi/lo split ~ f32 accurate).
  * BETA term needs pairwise distances: PE computes, into one PSUM accum,
        C' = x_m.x_n - 0.5*sq_m - 0.5*sq_n - (BIG/2)*y
    using 18 augmented contraction rows (sq / consts / +-A*onehot where
    A^2 = BIG/2).  Then dist' = sqrt(-2*C') = sqrt(d2 + BIG*y): same-class
    pairs (incl. the diagonal) land at ~1000 >> MARGIN so the (1-y) mask is
    free, and the sqrt input is always positive.  hinge^2 = min(dist-2, 0)^2.
  * Symmetry: only compute column range [row_block_start, L).  The 128-wide
    diagonal block counts once (contains both (i,j) and (j,i)); everything
    right of it counts twice (folded into the square scale).
  * Elementwise pipeline per 1024-wide piece: ACT sqrt (PSUM src) -> DVE
    v=min(dist-2,0) -> square+row-sum split between ACT (Square w/ accum_out)
    and DVE (mult + reduce) to balance engine load.  Partition reduce via a
    ones-vector matmul.  Host sums 8 scalars / (n*L*L).
"""

import math
import os

import numpy as np
import ml_dtypes

import concourse.bass as bass
import concourse.tile as tile
from concourse import bacc, mybir
import concourse.bass_utils as _bu
from concourse.bass_utils import run_bass_kernel_spmd

L = 2048
D = 512
NCLS = 16
P = 128
NCORES = 8
ALPHA = 0.1
BETA = 0.3
MARGIN = 2.0
BIGHALF = 5.0e5          # (BIG/2); BIG shift = 1e6 >> MARGIN^2
AAMP = math.sqrt(BIGHALF)
NAUG = 18
PIECE = 1024             # PSUM piece width (2 banks)

BF = mybir.dt.bfloat16
F32 = mybir.dt.float32
F8 = mybir.dt.float8e4

_NC_CACHE = {}


def _build_nc():
    # NOTE: --enable-ldw-opt=true was tried to elide per-matmul LDWEIGHTS;
    # walrus codegen crashes in visitInstLdweights, so it stays off.
    nc = bacc.Bacc(None, target_bir_lowering=False)

    # x^T packed for fp8 DoubleRow: xt8_j[p, s, l] = fp8(x[l, 256j+128s+p])
    xt8_d = [
        nc.dram_tensor(f"xt8{j}", [P, 2, L], F8, kind="ExternalInput")
        for j in range(2)
    ]
    # packed [xhi | onehot] rows for the class-aggregation matmuls
    xho_d = nc.dram_tensor("xho", [L, D + NCLS], BF, kind="ExternalInput")
    augl_d = nc.dram_tensor("augl", [NAUG, L], BF, kind="ExternalInput")
    augr_d = nc.dram_tensor("augr", [NAUG, L], BF, kind="ExternalInput")
    sqw_d = nc.dram_tensor("sqw", [P, L // P], F32, kind="ExternalInput")
    wcv_d = nc.dram_tensor("wcv", [P, L // P], F32, kind="ExternalInput")
    out_d = nc.dram_tensor("out", [1, 1], F32, kind="ExternalOutput")

    KT = D // P          # 4 contraction tiles
    LT = L // P          # 16 row blocks

    # enumerate pieces/segments up front (for accumulator sizing).
    # Row blocks processed in DESCENDING bi order: block bi only reads
    # xt columns [128*bi, L), so compute can start as soon as the top
    # column chunks have landed (xt is DMA'd last-columns-first).
    # bi=15 (a single 128-wide piece) is deferred to the very end to keep
    # the serial drain after the last matmul short.
    pieces = []          # (bi, c0, n, segs) ; segs: (kind, s0, s1)
    n_acc1 = 0
    n_acc2 = 0           # ACT-side weight-2 segments (scaled 2*BETA already)
    n_acc3 = 0           # DVE-side weight-2 segments (raw sum(v^2))
    w2_toggle = 0
    for bi in [*range(LT - 2, 4, -1), 0, 1, 2, 3, 4, LT - 1]:
        r0 = P * bi
        strip = L - r0
        for j in range((strip + PIECE - 1) // PIECE):
            c0 = r0 + PIECE * j
            n = min(PIECE, L - c0)
            segs = []
            if j == 0:
                segs.append(("diag", 0, P))
                if n > P:
                    segs.append(("w2", P, n))
            else:
                segs.append(("w2", 0, n))
            pieces.append((bi, c0, n, segs))
    with tile.TileContext(nc) as tc:
        with (
            tc.tile_pool(name="consts", bufs=1) as cp,
            tc.tile_pool(name="work", bufs=3) as wp,
            tc.tile_pool(name="accs", bufs=1) as ap,
            tc.tile_pool(name="pw_psum", bufs=3, space="PSUM") as pps,
            tc.tile_pool(name="sm_psum", bufs=1, space="PSUM") as sps,
        ):
            # ---- constant loads.  aug first (every piece needs it), then
            # xt in 256-col chunks, LAST columns first, so descending-bi
            # row blocks can start after the first few chunks land.
            # Tile's subtile deps gate each matmul on just the chunks it
            # reads.  classagg inputs go on the gpsimd queue. ----
            augl = cp.tile([NAUG, L], BF, tag="augl")
            nc.sync.dma_start(augl, augl_d[:])
            augr = cp.tile([NAUG, L], BF, tag="augr")
            nc.sync.dma_start(augr, augr_d[:])
            xt8 = [
                cp.tile([P, 2, L], F8, tag=f"xt8{j}", name=f"xt8{j}")
                for j in range(2)
            ]
            CCH = 256
            for c0 in range(L - CCH, -1, -CCH):
                for j in range(2):
                    nc.sync.dma_start(
                        xt8[j][:, :, c0 : c0 + CCH],
                        xt8_d[j][:, :, c0 : c0 + CCH],
                    )

            xho = []
            for k in range(LT):
                t = cp.tile([P, D + NCLS], BF, tag=f"xho{k}")
                nc.gpsimd.dma_start(t, xho_d[P * k : P * (k + 1), :])
                xho.append(t)
            sqw = cp.tile([P, L // P], F32, tag="sqw")
            nc.gpsimd.dma_start(sqw, sqw_d[:])
            wcv = cp.tile([P, L // P], F32, tag="wcv")
            nc.gpsimd.dma_start(wcv, wcv_d[:])
            ones = cp.tile([P, 1], F32, tag="ones")
            nc.vector.memset(ones, 1.0)

            # ---- class aggregation first: PE does useful work while the
            # big xt tiles stream in ----
            s_ps = sps.tile([NCLS, D], F32, tag="spsum")
            for k in range(LT):
                nc.tensor.matmul(
                    s_ps,
                    lhsT=xho[k][:, D : D + NCLS],
                    rhs=xho[k][:, :D],
                    start=(k == 0),
                    stop=(k == LT - 1),
                )

            # accumulator columns are assigned below while emitting
            acc1 = ap.tile([P, 16], F32, tag="acc1")
            acc2 = ap.tile([P, 24], F32, tag="acc2")
            acc3 = ap.tile([P, 24], F32, tag="acc3")
            fin = ap.tile([P, 8], F32, tag="fin")
            nc.vector.memset(fin, 0.0)
            nc.vector.memset(acc2, 0.0)
            nc.vector.memset(acc3, 0.0)

            # ---- pairwise upper-triangle strips ----
            for bi, c0, n, segs in pieces:
                r0 = P * bi
                pt = pps.tile([P, PIECE], F32, tag="pw")
                chunks = [
                    (ch0, min(ch0 + 512, n)) for ch0 in range(0, n, 512)
                ]
                # k-outer: consecutive matmuls share the same stationary
                # operand (lhsT), minimizing weight-reload pressure
                for k in range(2):
                    for ch0, ch1 in chunks:
                        nc.tensor.matmul(
                            pt[:, ch0:ch1],
                            lhsT=xt8[k][:, :, r0 : r0 + P],
                            rhs=xt8[k][:, :, c0 + ch0 : c0 + ch1],
                            start=(k == 0),
                            stop=False,
                            perf_mode=mybir.MatmulPerfMode.DoubleRow,
                        )
                for ch0, ch1 in chunks:
                    nc.tensor.matmul(
                        pt[:, ch0:ch1],
                        lhsT=augl[:, r0 : r0 + P],
                        rhs=augr[:, c0 + ch0 : c0 + ch1],
                        start=False,
                        stop=True,
                    )
                # dist = sqrt(-2*C') = sqrt(d2 + BIG*y)
                dist = wp.tile([P, PIECE], F32, tag="dist")
                nc.scalar.activation(
                    out=dist[:, :n],
                    in_=pt[:, :n],
                    func=mybir.ActivationFunctionType.Sqrt,
                    scale=-2.0,
                )
                # v = min(dist - 2, 0)  (== -hinge)
                v = wp.tile([P, PIECE], F32, tag="v")
                nc.vector.tensor_scalar(
                    out=v[:, :n],
                    in0=dist[:, :n],
                    scalar1=-float(MARGIN),
                    scalar2=0.0,
                    op0=mybir.AluOpType.add,
                    op1=mybir.AluOpType.min,
                )
                for kind, s0, s1 in segs:
                    if kind == "diag":
                        # weight-1 diagonal block on ACT, scaled by BETA
                        scr = wp.tile([P, PIECE], F32, tag="scr_act")
                        nc.scalar.activation(
                            out=scr[:, : s1 - s0],
                            in_=v[:, s0:s1],
                            func=mybir.ActivationFunctionType.Square,
                            scale=math.sqrt(BETA),
                            accum_out=acc1[:, n_acc1 : n_acc1 + 1],
                        )
                        n_acc1 += 1
                    elif w2_toggle == 2:
                        scr = wp.tile([P, PIECE], F32, tag="scr_act")
                        nc.scalar.activation(
                            out=scr[:, : s1 - s0],
                            in_=v[:, s0:s1],
                            func=mybir.ActivationFunctionType.Square,
                            scale=math.sqrt(2.0 * BETA),
                            accum_out=acc2[:, n_acc2 : n_acc2 + 1],
                        )
                        n_acc2 += 1
                        w2_toggle = 0
                    else:
                        scr = wp.tile([P, PIECE], F32, tag="scr_dve")
                        nc.vector.tensor_mul(
                            scr[:, : s1 - s0], v[:, s0:s1], v[:, s0:s1]
                        )
                        nc.vector.tensor_reduce(
                            out=acc3[:, n_acc3 : n_acc3 + 1],
                            in_=scr[:, : s1 - s0],
                            axis=mybir.AxisListType.X,
                            op=mybir.AluOpType.add,
                        )
                        n_acc3 += 1
                        w2_toggle += 1

            # ---- finish class aggregation ----
            s_sb = wp.tile([NCLS, D], F32, tag="s_sb")
            sper = ap.tile([NCLS, 1], F32, tag="sper")
            nc.scalar.activation(
                out=s_sb,
                in_=s_ps,
                func=mybir.ActivationFunctionType.Square,
                accum_out=sper,
            )
            # fin[:,1][0:16] = -2*ALPHA * sum_d s^2
            nc.scalar.mul(out=fin[:NCLS, 1:2], in_=sper, mul=-2.0 * ALPHA)
            # fin[:,0] = 2*ALPHA * sum_l cnt[class(l)] * sq[l]  (per partition)
            scr2 = ap.tile([P, L // P], F32, tag="scr2")
            nc.vector.tensor_mul(scr2, sqw, wcv)
            scr3 = ap.tile([P, 1], F32, tag="scr3")
            nc.vector.tensor_reduce(
                out=scr3, in_=scr2, axis=mybir.AxisListType.X,
                op=mybir.AluOpType.add,
            )
            nc.scalar.mul(out=fin[:, 0:1], in_=scr3, mul=2.0 * ALPHA)
            # fin[:,2] = sum acc1, fin[:,3] = sum acc2, fin[:,4] = 2B*sum acc3
            nc.vector.tensor_reduce(
                out=fin[:, 2:3], in_=acc1, axis=mybir.AxisListType.X,
                op=mybir.AluOpType.add,
            )
            nc.vector.tensor_reduce(
                out=fin[:, 3:4], in_=acc2, axis=mybir.AxisListType.X,
                op=mybir.AluOpType.add,
            )
            rt = ap.tile([P, 1], F32, tag="rt")
            nc.vector.tensor_reduce(
                out=rt, in_=acc3, axis=mybir.AxisListType.X,
                op=mybir.AluOpType.add,
            )
            nc.vector.tensor_scalar(
                out=fin[:, 4:5], in0=rt, scalar1=2.0 * BETA, scalar2=None,
                op0=mybir.AluOpType.mult,
            )

            # ---- partition reduce + output ----
            tot = sps.tile([1, 8], F32, tag="tot")
            nc.tensor.matmul(tot, lhsT=ones, rhs=fin, start=True, stop=True)
            res = ap.tile([1, 1], F32, tag="res")
            nc.vector.tensor_reduce(
                out=res, in_=tot, axis=mybir.AxisListType.X, op=mybir.AluOpType.add
            )
            nc.sync.dma_start(out_d[:], res)

    assert n_acc1 <= 16 and n_acc2 <= 24 and n_acc3 <= 24, (
        n_acc1, n_acc2, n_acc3,
    )
    nc.compile()
    return nc


def get_nc():
    if "nc" not in _NC_CACHE:
        _NC_CACHE["nc"] = _build_nc()
    return _NC_CACHE["nc"]


def prepare_in_maps(prediction, target_seg):
    """Host-side shard + layout prep. prediction [8,2048,512] f32,
    target_seg [8,2048] int."""
    prediction = np.asarray(prediction, dtype=np.float32)
    target_seg = np.asarray(target_seg)
    bf16 = ml_dtypes.bfloat16
    in_maps = []
    for i in range(NCORES):
        x = prediction[i]                        # [L, D] f32
        t = target_seg[i].astype(np.int64)       # [L]
        sq = np.square(x, dtype=np.float64).sum(axis=1)      # [L] f64
        sq32 = sq.astype(np.float32)
        ohf = (t[:, None] == np.arange(NCLS)[None, :]).astype(np.float32)

        xho = np.concatenate([x, ohf], axis=1).astype(bf16)
        # [D, L] -> two [128, 2, L] fp8 packs (k = 256j + 128s + p)
        xtT = np.ascontiguousarray(x.T).astype(ml_dtypes.float8_e4m3)
        xt8 = [
            np.ascontiguousarray(
                xtT[256 * j : 256 * (j + 1)].reshape(2, P, L).transpose(1, 0, 2)
            )
            for j in range(2)
        ]

        augl = np.zeros((NAUG, L), np.float32)
        augr = np.zeros((NAUG, L), np.float32)
        augl[:NCLS] = AAMP * ohf.T
        augr[:NCLS] = -AAMP * ohf.T
        augl[16] = sq32
        augr[16] = -0.5
        augl[17] = -0.5
        augr[17] = sq32

        cnt = np.bincount(t, minlength=NCLS).astype(np.float32)
        wc = cnt[t]                              # [L] count of own class

        in_maps.append(
            {
                "xt80": xt8[0],
                "xt81": xt8[1],
                "xho": xho,
                "augl": augl.astype(bf16),
                "augr": augr.astype(bf16),
                "sqw": np.ascontiguousarray(sq32.reshape(L // P, P).T),
                "wcv": np.ascontiguousarray(wc.reshape(L // P, P).T),
            }
        )
    return in_maps


def combine_outputs(results):
    total = sum(float(r["out"][0, 0]) for r in results)
    return np.asarray(total / (NCORES * L * L), dtype=np.float32)


def kernel(prediction, target_seg):
    nc = get_nc()
    in_maps = prepare_in_maps(prediction, target_seg)
    r = run_bass_kernel_spmd(nc, in_maps, core_ids=list(range(NCORES)))
    return combine_outputs(r.results)


# revision 20
# speedup vs baseline: 1.1485x; 1.1485x over previous
"""Trainium2 Bass kernel for nn_ContrastiveLoss (8-core data-parallel).

reference math (per batch n, L=2048, d=512):
    y[i,j]  = (seg[i] == seg[j])
    d2      = max(||x_i||^2 + ||x_j||^2 - 2 x_i.x_j, 0)
    dist    = sqrt(d2)
    hinge   = max(MARGIN - dist, 0)
    loss    = mean(ALPHA*y*d2 + BETA*(1-y)*hinge^2)

Strategy (one batch per NeuronCore):
  * ALPHA term via exact class aggregation:
        sum_ij y*d2 = sum_c [2*cnt_c*sum_{i in c}||x_i||^2 - 2*||sum_{i in c}x_i||^2]
    computed on-device with one-hot matmuls (bf16 hi/lo split ~ f32 accurate).
  * BETA term needs pairwise distances: PE computes, into one PSUM accum,
        C' = x_m.x_n - 0.5*sq_m - 0.5*sq_n - (BIG/2)*y
    using 18 augmented contraction rows (sq / consts / +-A*onehot where
    A^2 = BIG/2).  Then dist' = sqrt(-2*C') = sqrt(d2 + BIG*y): same-class
    pairs (incl. the diagonal) land at ~1000 >> MARGIN so the (1-y) mask is
    free, and the sqrt input is always positive.  hinge^2 = min(dist-2, 0)^2.
  * Symmetry: only compute column range [row_block_start, L).  The 128-wide
    diagonal block counts once (contains both (i,j) and (j,i)); everything
    right of it counts twice (folded into the square scale).
  * Elementwise pipeline per 1024-wide piece: ACT sqrt (PSUM src) -> DVE
    v=min(dist-2,0) -> square+row-sum split between ACT (Square w/ accum_out)
    and DVE (mult + reduce) to balance engine load.  Partition reduce via a
    ones-vector matmul.  Host sums 8 scalars / (n*L*L).
"""

import math
import os

import numpy as np
import ml_dtypes

import concourse.bass as bass
import concourse.tile as tile
from concourse import bacc, mybir
import concourse.bass_utils as _bu
from concourse.bass_utils import run_bass_kernel_spmd

L = 2048
D = 512
NCLS = 16
P = 128
NCORES = 8
ALPHA = 0.1
BETA = 0.3
MARGIN = 2.0
BIGHALF = 5.0e5          # (BIG/2); BIG shift = 1e6 >> MARGIN^2
AAMP = math.sqrt(BIGHALF)
NAUG = 18
PIECE = 1024             # PSUM piece width (2 banks)

BF = mybir.dt.bfloat16
F32 = mybir.dt.float32
F8 = mybir.dt.float8e4

_NC_CACHE = {}


def _build_nc():
    # NOTE: --enable-ldw-opt=true was tried to elide per-matmul LDWEIGHTS;
    # walrus codegen crashes in visitInstLdweights, so it stays off.
    nc = bacc.Bacc(None, target_bir_lowering=False)

    xt_d = nc.dram_tensor("xt", [D, L], BF, kind="ExternalInput")
    # packed [xhi | onehot] rows for the class-aggregation matmuls
    xho_d = nc.dram_tensor("xho", [L, D + NCLS], BF, kind="ExternalInput")
    augl_d = nc.dram_tensor("augl", [NAUG, L], BF, kind="ExternalInput")
    augr_d = nc.dram_tensor("augr", [NAUG, L], BF, kind="ExternalInput")
    sqw_d = nc.dram_tensor("sqw", [P, L // P], F32, kind="ExternalInput")
    wcv_d = nc.dram_tensor("wcv", [P, L // P], F32, kind="ExternalInput")
    out_d = nc.dram_tensor("out", [1, 1], F32, kind="ExternalOutput")

    KT = D // P          # 4 contraction tiles
    LT = L // P          # 16 row blocks

    # enumerate pieces/segments up front (for accumulator sizing).
    # Row blocks processed in DESCENDING bi order: block bi only reads
    # xt columns [128*bi, L), so compute can start as soon as the top
    # column chunks have landed (xt is DMA'd last-columns-first).
    # bi=15 (a single 128-wide piece) is deferred to the very end to keep
    # the serial drain after the last matmul short.
    pieces = []          # (bi, c0, n, segs) ; segs: (kind, s0, s1)
    n_acc1 = 0
    n_acc2 = 0           # ACT-side weight-2 segments (scaled 2*BETA already)
    n_acc3 = 0           # DVE-side weight-2 segments (raw sum(v^2))
    w2_toggle = 0
    for bi in [*range(LT - 2, 4, -1), 0, 1, 2, 3, 4, LT - 1]:
        r0 = P * bi
        strip = L - r0
        for j in range((strip + PIECE - 1) // PIECE):
            c0 = r0 + PIECE * j
            n = min(PIECE, L - c0)
            segs = []
            if j == 0:
                segs.append(("diag", 0, P))
                if n > P:
                    segs.append(("w2", P, n))
            else:
                segs.append(("w2", 0, n))
            pieces.append((bi, c0, n, segs))
    with tile.TileContext(nc) as tc:
        with (
            tc.tile_pool(name="consts", bufs=1) as cp,
            tc.tile_pool(name="work", bufs=3) as wp,
            tc.tile_pool(name="accs", bufs=1) as ap,
            tc.tile_pool(name="pw_psum", bufs=3, space="PSUM") as pps,
            tc.tile_pool(name="sm_psum", bufs=1, space="PSUM") as sps,
        ):
            # ---- constant loads.  aug first (every piece needs it), then
            # xt in 256-col chunks, LAST columns first, so descending-bi
            # row blocks can start after the first few chunks land.
            # Tile's subtile deps gate each matmul on just the chunks it
            # reads.  classagg inputs go on the gpsimd queue. ----
            augl = cp.tile([NAUG, L], BF, tag="augl")
            nc.sync.dma_start(augl, augl_d[:])
            augr = cp.tile([NAUG, L], BF, tag="augr")
            nc.sync.dma_start(augr, augr_d[:])
            xt = [
                cp.tile([P, L], BF, tag=f"xt{k}", name=f"xt{k}")
                for k in range(KT)
            ]
            CCH = 256
            for c0 in range(L - CCH, -1, -CCH):
                for k in range(KT):
                    nc.sync.dma_start(
                        xt[k][:, c0 : c0 + CCH],
                        xt_d[P * k : P * (k + 1), c0 : c0 + CCH],
                    )

            xho = []
            for k in range(LT):
                t = cp.tile([P, D + NCLS], BF, tag=f"xho{k}")
                nc.gpsimd.dma_start(t, xho_d[P * k : P * (k + 1), :])
                xho.append(t)
            sqw = cp.tile([P, L // P], F32, tag="sqw")
            nc.gpsimd.dma_start(sqw, sqw_d[:])
            wcv = cp.tile([P, L // P], F32, tag="wcv")
            nc.gpsimd.dma_start(wcv, wcv_d[:])
            ones = cp.tile([P, 1], F32, tag="ones")
            nc.vector.memset(ones, 1.0)

            # ---- class aggregation first: PE does useful work while the
            # big xt tiles stream in ----
            s_ps = sps.tile([NCLS, D], F32, tag="spsum")
            for k in range(LT):
                nc.tensor.matmul(
                    s_ps,
                    lhsT=xho[k][:, D : D + NCLS],
                    rhs=xho[k][:, :D],
                    start=(k == 0),
                    stop=(k == LT - 1),
                )

            # accumulator columns are assigned below while emitting
            acc1 = ap.tile([P, 16], F32, tag="acc1")
            acc2 = ap.tile([P, 24], F32, tag="acc2")
            acc3 = ap.tile([P, 24], F32, tag="acc3")
            fin = ap.tile([P, 8], F32, tag="fin")
            nc.vector.memset(fin, 0.0)
            nc.vector.memset(acc2, 0.0)
            nc.vector.memset(acc3, 0.0)

            # ---- pairwise upper-triangle strips ----
            for bi, c0, n, segs in pieces:
                r0 = P * bi
                pt = pps.tile([P, PIECE], F32, tag="pw")
                chunks = [
                    (ch0, min(ch0 + 512, n)) for ch0 in range(0, n, 512)
                ]
                # k-outer: consecutive matmuls share the same stationary
                # operand (lhsT), minimizing weight-reload pressure
                for k in range(KT):
                    for ch0, ch1 in chunks:
                        nc.tensor.matmul(
                            pt[:, ch0:ch1],
                            lhsT=xt[k][:, r0 : r0 + P],
                            rhs=xt[k][:, c0 + ch0 : c0 + ch1],
                            start=(k == 0),
                            stop=False,
                        )
                for ch0, ch1 in chunks:
                    nc.tensor.matmul(
                        pt[:, ch0:ch1],
                        lhsT=augl[:, r0 : r0 + P],
                        rhs=augr[:, c0 + ch0 : c0 + ch1],
                        start=False,
                        stop=True,
                    )
                # dist = sqrt(-2*C') = sqrt(d2 + BIG*y)
                dist = wp.tile([P, PIECE], F32, tag="dist")
                nc.scalar.activation(
                    out=dist[:, :n],
                    in_=pt[:, :n],
                    func=mybir.ActivationFunctionType.Sqrt,
                    scale=-2.0,
                )
                # v = min(dist - 2, 0)  (== -hinge)
                v = wp.tile([P, PIECE], F32, tag="v")
                nc.vector.tensor_scalar(
                    out=v[:, :n],
                    in0=dist[:, :n],
                    scalar1=-float(MARGIN),
                    scalar2=0.0,
                    op0=mybir.AluOpType.add,
                    op1=mybir.AluOpType.min,
                )
                for kind, s0, s1 in segs:
                    if kind == "diag":
                        # weight-1 diagonal block on ACT, scaled by BETA
                        scr = wp.tile([P, PIECE], F32, tag="scr_act")
                        nc.scalar.activation(
                            out=scr[:, : s1 - s0],
                            in_=v[:, s0:s1],
                            func=mybir.ActivationFunctionType.Square,
                            scale=math.sqrt(BETA),
                            accum_out=acc1[:, n_acc1 : n_acc1 + 1],
                        )
                        n_acc1 += 1
                    elif w2_toggle == 2:
                        scr = wp.tile([P, PIECE], F32, tag="scr_act")
                        nc.scalar.activation(
                            out=scr[:, : s1 - s0],
                            in_=v[:, s0:s1],
                            func=mybir.ActivationFunctionType.Square,
                            scale=math.sqrt(2.0 * BETA),
                            accum_out=acc2[:, n_acc2 : n_acc2 + 1],
                        )
                        n_acc2 += 1
                        w2_toggle = 0
                    else:
                        scr = wp.tile([P, PIECE], F32, tag="scr_dve")
                        nc.vector.tensor_mul(
                            scr[:, : s1 - s0], v[:, s0:s1], v[:, s0:s1]
                        )
                        nc.vector.tensor_reduce(
                            out=acc3[:, n_acc3 : n_acc3 + 1],
                            in_=scr[:, : s1 - s0],
                            axis=mybir.AxisListType.X,
                            op=mybir.AluOpType.add,
                        )
                        n_acc3 += 1
                        w2_toggle += 1

            # ---- finish class aggregation ----
            s_sb = wp.tile([NCLS, D], F32, tag="s_sb")
            sper = ap.tile([NCLS, 1], F32, tag="sper")
            nc.scalar.activation(
                out=s_sb,
                in_=s_ps,
                func=mybir.ActivationFunctionType.Square,
                accum_out=sper,
            )
            # fin[:,1][0:16] = -2*ALPHA * sum_d s^2
            nc.scalar.mul(out=fin[:NCLS, 1:2], in_=sper, mul=-2.0 * ALPHA)
            # fin[:,0] = 2*ALPHA * sum_l cnt[class(l)] * sq[l]  (per partition)
            scr2 = ap.tile([P, L // P], F32, tag="scr2")
            nc.vector.tensor_mul(scr2, sqw, wcv)
            scr3 = ap.tile([P, 1], F32, tag="scr3")
            nc.vector.tensor_reduce(
                out=scr3, in_=scr2, axis=mybir.AxisListType.X,
                op=mybir.AluOpType.add,
            )
            nc.scalar.mul(out=fin[:, 0:1], in_=scr3, mul=2.0 * ALPHA)
            # fin[:,2] = sum acc1, fin[:,3] = sum acc2, fin[:,4] = 2B*sum acc3
            nc.vector.tensor_reduce(
                out=fin[:, 2:3], in_=acc1, axis=mybir.AxisListType.X,
                op=mybir.AluOpType.add,
            )
            nc.vector.tensor_reduce(
                out=fin[:, 3:4], in_=acc2, axis=mybir.AxisListType.X,
                op=mybir.AluOpType.add,
            )
            rt = ap.tile([P, 1], F32, tag="rt")
            nc.vector.tensor_reduce(
                out=rt, in_=acc3, axis=mybir.AxisListType.X,
                op=mybir.AluOpType.add,
            )
            nc.vector.tensor_scalar(
                out=fin[:, 4:5], in0=rt, scalar1=2.0 * BETA, scalar2=None,
                op0=mybir.AluOpType.mult,
            )

            # ---- partition reduce + output ----
            tot = sps.tile([1, 8], F32, tag="tot")
            nc.tensor.matmul(tot, lhsT=ones, rhs=fin, start=True, stop=True)
            res = ap.tile([1, 1], F32, tag="res")
            nc.vector.tensor_reduce(
                out=res, in_=tot, axis=mybir.AxisListType.X, op=mybir.AluOpType.add
            )
            nc.sync.dma_start(out_d[:], res)

    assert n_acc1 <= 16 and n_acc2 <= 24 and n_acc3 <= 24, (
        n_acc1, n_acc2, n_acc3,
    )
    nc.compile()
    return nc


def get_nc():
    if "nc" not in _NC_CACHE:
        _NC_CACHE["nc"] = _build_nc()
    return _NC_CACHE["nc"]


def prepare_in_maps(prediction, target_seg):
    """Host-side shard + layout prep. prediction [8,2048,512] f32,
    target_seg [8,2048] int."""
    prediction = np.asarray(prediction, dtype=np.float32)
    target_seg = np.asarray(target_seg)
    bf16 = ml_dtypes.bfloat16
    in_maps = []
    for i in range(NCORES):
        x = prediction[i]                        # [L, D] f32
        t = target_seg[i].astype(np.int64)       # [L]
        sq = np.square(x, dtype=np.float64).sum(axis=1)      # [L] f64
        sq32 = sq.astype(np.float32)
        ohf = (t[:, None] == np.arange(NCLS)[None, :]).astype(np.float32)

        xho = np.concatenate([x, ohf], axis=1).astype(bf16)

        augl = np.zeros((NAUG, L), np.float32)
        augr = np.zeros((NAUG, L), np.float32)
        augl[:NCLS] = AAMP * ohf.T
        augr[:NCLS] = -AAMP * ohf.T
        augl[16] = sq32
        augr[16] = -0.5
        augl[17] = -0.5
        augr[17] = sq32

        cnt = np.bincount(t, minlength=NCLS).astype(np.float32)
        wc = cnt[t]                              # [L] count of own class

        in_maps.append(
            {
                "xt": np.ascontiguousarray(x.T).astype(bf16),
                "xho": xho,
                "augl": augl.astype(bf16),
                "augr": augr.astype(bf16),
                "sqw": np.ascontiguousarray(sq32.reshape(L // P, P).T),
                "wcv": np.ascontiguousarray(wc.reshape(L // P, P).T),
            }
        )
    return in_maps


def combine_outputs(results):
    total = sum(float(r["out"][0, 0]) for r in results)
    return np.asarray(total / (NCORES * L * L), dtype=np.float32)


def kernel(prediction, target_seg):
    nc = get_nc()
    in_maps = prepare_in_maps(prediction, target_seg)
    r = run_bass_kernel_spmd(nc, in_maps, core_ids=list(range(NCORES)))
    return combine_outputs(r.results)
